# revision 26
# baseline (speedup 1.0000x reference)
"""Trainium2 kernel for nn_PolynomialLayer: out = [x, x_i*x_j (i<=j)] @ W.T + bias.

Data-parallel over batch across 8 NeuronCores. Each core computes
out^T[512, 1024] = sum_c Wc^T @ Pc + bias for 66 feature chunks
(1 linear + 1 squares + 64 partition-rotation chunks).

Structure (v2):
  - The last POLY_NFP8 (default 16) chunks run as fp8-e4m3 DoubleRow
    matmul pairs (2 K-tiles per pass, ~1.4x tensor-engine throughput);
    the rest are bf16. 16 fp8 chunks keep rel_fro error ~1.8e-2 < 2e-2.
  - Wavefront bank schedule: the 8 PSUM banks (4 n-chunks x 2 b-halves)
    process unit u at wave position u+i, so bank i finishes its K
    accumulation i units early and its bias-add copy + output DMA piece
    overlap the remaining matmul stream.
  - Output is written bf16 (adds ~1e-3 rel error, halves outbound DMA);
    host casts back to f32.
  - Startup-critical DMAs (xT, first weights, fp8 weights) trigger on the
    scalar HW-DGE queue in parallel with the sync-queue trigger stream;
    descriptors round-robin over all 16 DMA engines either way.
"""

import os
import sys
import numpy as np

for _p in ("/opt/trn_rl_repo",):
    if os.path.isdir(_p) and _p not in sys.path:
        sys.path.append(_p)

B, D, NOUT = 8192, 128, 512
NCORES = 8
BC = B // NCORES            # 1024 batch rows per core
NCHUNK = 66                 # 1 linear + 1 squares + 64 rotation chunks
NROT = 64                   # rotation distances d=1..64
NN = NOUT // 128            # output partition chunks (4)
NB = BC // 512              # moving-operand chunks per core (2)
NBANK = NN * NB             # PSUM banks; bank i -> (b=i//4, n=i%4)

NFP8 = int(os.environ.get("POLY_NFP8", "16"))   # trailing chunks in fp8 (even)
assert NFP8 % 2 == 0 and 0 <= NFP8 <= 32
NBF = NCHUNK - NFP8         # leading bf16 chunks
NP8 = NFP8 // 2             # fp8 DoubleRow pairs
NU = NBF + NP8              # wavefront units
DMA_ENG = os.environ.get("POLY_DMA_ENG", "sync")


def _ensure_axon_hooks_stub():
    """concourse's trace path imports antenv.axon_hooks; provide a stub if
    this image lacks it so an env-triggered trace degrades instead of
    crashing."""
    try:
        import antenv.axon_hooks  # noqa: F401
        return
    except Exception:
        pass
    try:
        import types
        import antenv
        m = types.ModuleType("antenv.axon_hooks")
        m._hook = None
        m.set_axon_ntff_profile_hook = lambda h: setattr(m, "_hook", h)
        m.get_axon_ntff_profile_hook = lambda: m._hook
        sys.modules["antenv.axon_hooks"] = m
        antenv.axon_hooks = m
    except Exception:
        pass


def _pair_index_map():
    """Map (chunk c, partition p) -> column index in the reference feature
    order (or -1 for padding).

    Reference order: [x_0..x_127] then pairs (i,j) i<=j in
    combinations_with_replacement order.
    Chunk layout: c=0 linear; c=1 squares; c=2..65 -> d=c-1 in 1..64 with
    (i,j) = sorted(p, (p+d) % 128); for d=64 only p<64 is valid.
    """
    idx = np.full((NCHUNK, D), -1, dtype=np.int64)
    off = 128 * np.arange(D) - (np.arange(D) * (np.arange(D) - 1)) // 2

    def pair_idx(i, j):
        return D + off[i] + (j - i)

    idx[0, :] = np.arange(D)
    p = np.arange(D)
    idx[1, :] = pair_idx(p, p)
    for d in range(1, NROT + 1):
        c = 1 + d
        q = (p + d) % D
        i = np.minimum(p, q)
        j = np.maximum(p, q)
        v = pair_idx(i, j)
        if d == NROT:
            v = np.where(p < 64, v, -1)
        idx[c, :] = v
    return idx


def _greedy_groups(total, lead=(1, 1, 2, 4), step=6):
    """Group sizes: small leading groups so the pipeline starts fast."""
    sizes = []
    s = 0
    for l in lead:
        if s + l > total:
            break
        sizes.append(l)
        s += l
    while total - s > step:
        sizes.append(step)
        s += step
    if total - s:
        sizes.append(total - s)
    assert sum(sizes) == total
    return sizes


_nc_cache = None


def _build_nc():
    global _nc_cache
    if _nc_cache is not None:
        return _nc_cache
    import concourse.tile as tile
    from concourse import bacc, mybir

    bf = mybir.dt.bfloat16
    f8 = mybir.dt.float8e4
    f32 = mybir.dt.float32

    nc = bacc.Bacc("TRN2", target_bir_lowering=False, debug=False)
    xT_ext = nc.dram_tensor("xT", [D, BC], bf, kind="ExternalInput")
    rots_ext = nc.dram_tensor("rots", [D, NROT, BC], bf, kind="ExternalInput")
    wp_ext = nc.dram_tensor("wp", [D, NBF, NOUT], bf, kind="ExternalInput")
    if NP8:
        w8_ext = nc.dram_tensor("w8", [D, NP8, 2, NOUT], f8, kind="ExternalInput")
    bias_ext = nc.dram_tensor("biasp", [D, NN], f32, kind="ExternalInput")
    out_ext = nc.dram_tensor("out", [NOUT, BC], bf, kind="ExternalOutput")

    # bf16 weight groups over chunks 0..NBF-1
    wg_sizes = _greedy_groups(NBF, lead=(2, 2, 4))
    wg_starts = np.cumsum([0] + wg_sizes).tolist()
    wg_of_chunk = {}
    for g, s in enumerate(wg_starts[:-1]):
        for c in range(s, wg_starts[g + 1]):
            wg_of_chunk[c] = g
    # bf16 rotation groups over d=1..NBF-2 (rot index rc = d-1 in 0..NBF-3)
    NRBF = NBF - 2
    rg_sizes = _greedy_groups(NRBF, step=5)
    rg_starts = np.cumsum([0] + rg_sizes).tolist()
    rg_of_rc = {}
    for g, s in enumerate(rg_starts[:-1]):
        for r in range(s, rg_starts[g + 1]):
            rg_of_rc[r] = g
    # fp8 rotation streams arrive in 2 halves, triggered well before use
    r8_half = max(NP8 // 2, 1)

    with tile.TileContext(nc) as tc:
        with (
            tc.tile_pool(name="xpool", bufs=1) as xpool,
            tc.tile_pool(name="wpool", bufs=5) as wpool,
            tc.tile_pool(name="rpool", bufs=3) as rpool,
            tc.tile_pool(name="pbf", bufs=10) as pbf,
            tc.tile_pool(name="opool", bufs=6) as opool,
            tc.tile_pool(name="psum", bufs=1, space="PSUM") as psum,
            tc.tile_pool(name="w8pool", bufs=1) as w8pool,
            tc.tile_pool(name="r8pool", bufs=1) as r8pool,
            tc.tile_pool(name="pf8", bufs=max(min(NP8, 8), 1)) as pf8,
        ):
            dmae = getattr(nc, DMA_ENG)

            xT = xpool.tile([D, BC], bf)
            h = BC // 2
            # first transfers on the scalar HW-DGE queue: parallel with the
            # sync-queue triggers and nothing big competing for engines yet
            nc.scalar.dma_start(xT[:, 0:h], xT_ext[:, 0:h])

            ps = [psum.tile([D, 512], f32, name=f"ps_{i}", tag=f"ps_{i}")
                  for i in range(NBANK)]

            # PE clock warm-up: dummy matmuls into the last-started bank while
            # the first transfers are in flight (the p-state ramp needs ~3us
            # of continuous PE work to reach full clock)
            warm = xpool.tile([D, 512], bf, name="warm")
            nc.vector.memset(warm[:], 0.0)
            for _ in range(9):
                nc.tensor.matmul(ps[NBANK - 1][:], warm[:, 0:128], warm[:],
                                 start=True, stop=True)

            wg_tiles = {}
            rg_tiles = {}
            w8_tiles = {}
            r8_tiles = {}
            prod = {}     # unit -> (tile_or_ap, is_pair)

            # weight group 0 (covers unit 0) + second xT half up front
            wg0 = wpool.tile([D, wg_sizes[0] * NOUT], bf, name="wg", tag="wg")
            dmae.dma_start(wg0[:], wp_ext[:, 0:wg_sizes[0], :])
            wg_tiles[0] = wg0
            nc.scalar.dma_start(xT[:, h:BC], xT_ext[:, h:BC])
            bias = xpool.tile([D, NN], f32)
            w8t = w8pool.tile([D, NP8, 2, NOUT], f8, name='w8t') if NP8 else None

            def issue_unit_dmas(u):
                if u == 8:
                    dmae.dma_start(bias[:], bias_ext[:])
                if NP8 and u == 16:
                    dmae.dma_start(w8t[:], w8_ext[:])
                if NP8 and u == max(NBF - 10, 2):
                    rt = r8pool.tile([D, 2 * r8_half * BC], bf, name="r8", tag="r8")
                    dmae.dma_start(rt[:], rots_ext[:, NRBF:NRBF + 2 * r8_half, :])
                    r8_tiles[0] = rt
                if NP8 and u == max(NBF - 5, 3):
                    n2 = 2 * (NP8 - r8_half)
                    rt = r8pool.tile([D, n2 * BC], bf, name="r8b", tag="r8b")
                    dmae.dma_start(
                        rt[:], rots_ext[:, NRBF + 2 * r8_half:NRBF + 2 * r8_half + n2, :])
                    r8_tiles[1] = rt
                if u < NBF:
                    g = wg_of_chunk[u]
                    if g not in wg_tiles and u == wg_starts[g]:
                        sz = wg_sizes[g]
                        wg = wpool.tile([D, sz * NOUT], bf, name="wg", tag="wg")
                        dmae.dma_start(wg[:], wp_ext[:, u:u + sz, :])
                        wg_tiles[g] = wg
                    rc = u - 2
                    if rc >= 0:
                        rg = rg_of_rc[rc]
                        if rg not in rg_tiles and rc == rg_starts[rg]:
                            sz = rg_sizes[rg]
                            rt = rpool.tile([D, sz * BC], bf, name="rg", tag="rg")
                            dmae.dma_start(rt[:], rots_ext[:, rc:rc + sz, :])
                            rg_tiles[rg] = rt

            def compute_product(u):
                if u == 0:
                    prod[0] = (xT, False)
                    return
                if u < NBF:
                    pt = pbf.tile([D, BC], bf, name="pt", tag="pt")
                    if u == 1:
                        nc.vector.tensor_mul(pt[:, 0:h], xT[:, 0:h], xT[:, 0:h])
                        nc.vector.tensor_mul(pt[:, h:BC], xT[:, h:BC], xT[:, h:BC])
                    else:
                        rc = u - 2
                        g = rg_of_rc[rc]
                        roff = rc - rg_starts[g]
                        rt = rg_tiles[g]
                        nc.vector.tensor_mul(
                            pt[:], xT[:], rt[:, roff * BC:(roff + 1) * BC])
                    prod[u] = (pt, False)
                else:
                    j = u - NBF
                    g = 0 if j < r8_half else 1
                    joff = j - g * r8_half
                    rt = r8_tiles[g]
                    pt = pf8.tile([D, 2, BC], f8, name="p8", tag="p8")
                    for k in range(2):
                        rcol = (2 * joff + k) * BC
                        nc.vector.tensor_mul(
                            pt[:, k, :], xT[:], rt[:, rcol:rcol + BC])
                    prod[u] = (pt, True)

            def release_unit(u):
                # last consumer done: allow pool rings to recycle
                prod.pop(u, None)
                if u < NBF:
                    g = wg_of_chunk[u]
                    if u == wg_starts[g] + wg_sizes[g] - 1:
                        wg_tiles.pop(g, None)

            def emit_matmul(u, i):
                b, n = i // NN, i % NN
                if u < NBF:
                    g = wg_of_chunk[u]
                    wg = wg_tiles[g]
                    woff = (u - wg_starts[g]) * NOUT
                    mv, _ = prod[u]
                    nc.tensor.matmul(
                        ps[i][:],
                        wg[:, woff + n * 128:woff + (n + 1) * 128],
                        mv[:, b * 512:(b + 1) * 512],
                        start=(u == 0),
                        stop=(u == NU - 1),
                    )
                else:
                    j = u - NBF
                    pt, _ = prod[u]
                    nc.tensor.matmul(
                        ps[i][:],
                        w8t[:, j, :, n * 128:(n + 1) * 128],
                        pt[:, :, b * 512:(b + 1) * 512],
                        start=False,
                        stop=(u == NU - 1),
                        perf_mode=mybir.MatmulPerfMode.DoubleRow,
                    )

            def drain_bank(i):
                b, n = i // NN, i % NN
                ot = opool.tile([D, 512], bf, name=f"ot_{i}", tag=f"ot_{i}")
                if i % 2 == 0:
                    nc.scalar.activation(
                        ot[:], ps[i][:],
                        mybir.ActivationFunctionType.Identity,
                        bias=bias[:, n:n + 1],
                    )
                else:
                    nc.vector.tensor_scalar_add(ot[:], ps[i][:], bias[:, n:n + 1])
                nc.sync.dma_start(
                    out_ext[n * 128:(n + 1) * 128, b * 512:(b + 1) * 512],
                    ot[:],
                )

            # wavefront: bank i processes unit u at step s = u + i
            for s in range(NU + NBANK - 1):
                if s < NU:
                    issue_unit_dmas(s)
                    compute_product(s)
                for i in range(NBANK):
                    u = s - i
                    if 0 <= u < NU:
                        emit_matmul(u, i)
                if 1 <= s <= 6:
                    # bank 7 is free until its real start at step 7: a dummy
                    # group here fills any data-arrival bubble and keeps the
                    # PE clock from dropping back to the slow p-state
                    nc.tensor.matmul(ps[NBANK - 1][:], warm[:, 0:128], warm[:],
                                     start=True, stop=True)
                u_done = s - (NBANK - 1)
                if u_done >= 0:
                    release_unit(u_done)
                if s >= NU - 1:
                    drain_bank(s - (NU - 1))

    nc.compile()
    _nc_cache = nc
    return nc


def _prep_inputs(x, weights, bias):
    import ml_dtypes
    bf = np.dtype(ml_dtypes.bfloat16)
    f8 = np.dtype(ml_dtypes.float8_e4m3)

    x = np.asarray(x, dtype=np.float32)
    weights = np.asarray(weights, dtype=np.float32)
    bias = np.asarray(bias, dtype=np.float32)

    idx = _pair_index_map()
    wcols = weights.T  # [8384, 512]
    wp = np.zeros((NCHUNK, D, NOUT), dtype=np.float32)
    valid = idx >= 0
    wp[valid] = wcols[idx[valid]]
    wp_bf = np.ascontiguousarray(wp[:NBF].transpose(1, 0, 2)).astype(bf)
    if NP8:
        w8 = wp[NBF:].reshape(NP8, 2, D, NOUT).transpose(2, 0, 1, 3)
        w8 = np.ascontiguousarray(w8).astype(f8)

    biasp = np.ascontiguousarray(bias.reshape(NN, 128).T)  # [128, NN] f32

    in_maps = []
    for k in range(NCORES):
        xs = np.ascontiguousarray(x[k * BC:(k + 1) * BC].T).astype(bf)  # [128, BC]
        rots = np.stack([np.roll(xs, -d, axis=0) for d in range(1, NROT + 1)])
        rots = np.ascontiguousarray(rots.transpose(1, 0, 2))  # [D, NROT, BC]
        m = {"xT": xs, "rots": rots, "wp": wp_bf, "biasp": biasp}
        if NP8:
            m["w8"] = w8
        in_maps.append(m)
    return in_maps


def kernel(x, weights, bias):
    _ensure_axon_hooks_stub()
    from concourse.bass_utils import run_bass_kernel_spmd

    nc = _build_nc()
    in_maps = _prep_inputs(x, weights, bias)
    res = run_bass_kernel_spmd(nc, in_maps, core_ids=list(range(NCORES)))
    outT = np.concatenate(
        [np.asarray(res.results[k]["out"], dtype=np.float32) for k in range(NCORES)],
        axis=1,
    )
    out = np.ascontiguousarray(outT.T, dtype=np.float32)  # [8192, 512]
    kernel.last_results = res
    return out


# revision 27
# speedup vs baseline: 1.0155x; 1.0155x over previous
"""Trainium2 kernel for nn_PolynomialLayer: out = [x, x_i*x_j (i<=j)] @ W.T + bias.

Data-parallel over batch across 8 NeuronCores. Each core computes
out^T[512, 1024] = sum_c Wc^T @ Pc + bias for 66 feature chunks
(1 linear + 1 squares + 64 partition-rotation chunks).

Structure (v2):
  - The last POLY_NFP8 (default 16) chunks run as fp8-e4m3 DoubleRow
    matmul pairs (2 K-tiles per pass, ~1.4x tensor-engine throughput);
    the rest are bf16. 16 fp8 chunks keep rel_fro error ~1.8e-2 < 2e-2.
  - Wavefront bank schedule: the 8 PSUM banks (4 n-chunks x 2 b-halves)
    process unit u at wave position u+i, so bank i finishes its K
    accumulation i units early and its bias-add copy + output DMA piece
    overlap the remaining matmul stream.
  - Output is written bf16 (adds ~1e-3 rel error, halves outbound DMA);
    host casts back to f32.
  - Startup-critical DMAs (xT, first weights, fp8 weights) trigger on the
    scalar HW-DGE queue in parallel with the sync-queue trigger stream;
    descriptors round-robin over all 16 DMA engines either way.
"""

import os
import sys
import numpy as np

for _p in ("/opt/trn_rl_repo",):
    if os.path.isdir(_p) and _p not in sys.path:
        sys.path.append(_p)

B, D, NOUT = 8192, 128, 512
NCORES = 8
BC = B // NCORES            # 1024 batch rows per core
NCHUNK = 66                 # 1 linear + 1 squares + 64 rotation chunks
NROT = 64                   # rotation distances d=1..64
NN = NOUT // 128            # output partition chunks (4)
NB = BC // 512              # moving-operand chunks per core (2)
NBANK = NN * NB             # PSUM banks; bank i -> (b=i//4, n=i%4)

NFP8 = int(os.environ.get("POLY_NFP8", "16"))   # trailing chunks in fp8 (even)
assert NFP8 % 2 == 0 and 0 <= NFP8 <= 32
NBF = NCHUNK - NFP8         # leading bf16 chunks
NP8 = NFP8 // 2             # fp8 DoubleRow pairs
NU = NBF + NP8              # wavefront units
DMA_ENG = os.environ.get("POLY_DMA_ENG", "sync")


def _ensure_axon_hooks_stub():
    """concourse's trace path imports antenv.axon_hooks; provide a stub if
    this image lacks it so an env-triggered trace degrades instead of
    crashing."""
    try:
        import antenv.axon_hooks  # noqa: F401
        return
    except Exception:
        pass
    try:
        import types
        import antenv
        m = types.ModuleType("antenv.axon_hooks")
        m._hook = None
        m.set_axon_ntff_profile_hook = lambda h: setattr(m, "_hook", h)
        m.get_axon_ntff_profile_hook = lambda: m._hook
        sys.modules["antenv.axon_hooks"] = m
        antenv.axon_hooks = m
    except Exception:
        pass


def _pair_index_map():
    """Map (chunk c, partition p) -> column index in the reference feature
    order (or -1 for padding).

    Reference order: [x_0..x_127] then pairs (i,j) i<=j in
    combinations_with_replacement order.
    Chunk layout: c=0 linear; c=1 squares; c=2..65 -> d=c-1 in 1..64 with
    (i,j) = sorted(p, (p+d) % 128); for d=64 only p<64 is valid.
    """
    idx = np.full((NCHUNK, D), -1, dtype=np.int64)
    off = 128 * np.arange(D) - (np.arange(D) * (np.arange(D) - 1)) // 2

    def pair_idx(i, j):
        return D + off[i] + (j - i)

    idx[0, :] = np.arange(D)
    p = np.arange(D)
    idx[1, :] = pair_idx(p, p)
    for d in range(1, NROT + 1):
        c = 1 + d
        q = (p + d) % D
        i = np.minimum(p, q)
        j = np.maximum(p, q)
        v = pair_idx(i, j)
        if d == NROT:
            v = np.where(p < 64, v, -1)
        idx[c, :] = v
    return idx


def _greedy_groups(total, lead=(1, 1, 2, 4), step=6):
    """Group sizes: small leading groups so the pipeline starts fast."""
    sizes = []
    s = 0
    for l in lead:
        if s + l > total:
            break
        sizes.append(l)
        s += l
    while total - s > step:
        sizes.append(step)
        s += step
    if total - s:
        sizes.append(total - s)
    assert sum(sizes) == total
    return sizes


_nc_cache = None


def _build_nc():
    global _nc_cache
    if _nc_cache is not None:
        return _nc_cache
    import concourse.tile as tile
    from concourse import bacc, mybir

    bf = mybir.dt.bfloat16
    f8 = mybir.dt.float8e4
    f32 = mybir.dt.float32

    nc = bacc.Bacc("TRN2", target_bir_lowering=False, debug=False)
    xT_ext = nc.dram_tensor("xT", [D, BC], bf, kind="ExternalInput")
    rots_ext = nc.dram_tensor("rots", [D, NROT, BC], bf, kind="ExternalInput")
    wp_ext = nc.dram_tensor("wp", [D, NBF, NOUT], bf, kind="ExternalInput")
    if NP8:
        w8_ext = nc.dram_tensor("w8", [D, NP8, 2, NOUT], f8, kind="ExternalInput")
    bias_ext = nc.dram_tensor("biasp", [D, NN], f32, kind="ExternalInput")
    out_ext = nc.dram_tensor("out", [NOUT, BC], bf, kind="ExternalOutput")

    # bf16 weight groups over chunks 0..NBF-1
    wg_sizes = _greedy_groups(NBF, lead=(2, 2, 4))
    wg_starts = np.cumsum([0] + wg_sizes).tolist()
    wg_of_chunk = {}
    for g, s in enumerate(wg_starts[:-1]):
        for c in range(s, wg_starts[g + 1]):
            wg_of_chunk[c] = g
    # bf16 rotation groups over d=1..NBF-2 (rot index rc = d-1 in 0..NBF-3)
    NRBF = NBF - 2
    rg_sizes = _greedy_groups(NRBF, step=5)
    rg_starts = np.cumsum([0] + rg_sizes).tolist()
    rg_of_rc = {}
    for g, s in enumerate(rg_starts[:-1]):
        for r in range(s, rg_starts[g + 1]):
            rg_of_rc[r] = g
    # fp8 rotation streams arrive in 2 halves, triggered well before use
    r8_half = max(NP8 // 2, 1)

    with tile.TileContext(nc) as tc:
        with (
            tc.tile_pool(name="xpool", bufs=1) as xpool,
            tc.tile_pool(name="wpool", bufs=5) as wpool,
            tc.tile_pool(name="rpool", bufs=3) as rpool,
            tc.tile_pool(name="pbf", bufs=10) as pbf,
            tc.tile_pool(name="opool", bufs=6) as opool,
            tc.tile_pool(name="psum", bufs=1, space="PSUM") as psum,
            tc.tile_pool(name="w8pool", bufs=1) as w8pool,
            tc.tile_pool(name="r8pool", bufs=1) as r8pool,
            tc.tile_pool(name="pf8", bufs=max(min(NP8, 8), 1)) as pf8,
        ):
            dmae = getattr(nc, DMA_ENG)

            xT = xpool.tile([D, BC], bf)
            h = BC // 2
            # first transfers on the scalar HW-DGE queue: parallel with the
            # sync-queue triggers and nothing big competing for engines yet
            nc.scalar.dma_start(xT[:, 0:h], xT_ext[:, 0:h])

            ps = [psum.tile([D, 512], f32, name=f"ps_{i}", tag=f"ps_{i}")
                  for i in range(NBANK)]

            # PE clock warm-up: dummy matmuls into the last-started bank while
            # the first transfers are in flight (the p-state ramp needs ~3us
            # of continuous PE work to reach full clock)
            warm = xpool.tile([D, 512], bf, name="warm")
            nc.vector.memset(warm[:], 0.0)
            for _ in range(9):
                nc.tensor.matmul(ps[NBANK - 1][:], warm[:, 0:128], warm[:],
                                 start=True, stop=True)

            wg_tiles = {}
            rg_tiles = {}
            w8_tiles = {}
            r8_tiles = {}
            prod = {}     # unit -> (tile_or_ap, is_pair)

            # weight group 0 (covers unit 0) + second xT half up front
            wg0 = wpool.tile([D, wg_sizes[0] * NOUT], bf, name="wg", tag="wg")
            dmae.dma_start(wg0[:], wp_ext[:, 0:wg_sizes[0], :])
            wg_tiles[0] = wg0
            nc.scalar.dma_start(xT[:, h:BC], xT_ext[:, h:BC])
            bias = xpool.tile([D, NN], f32)
            w8t = w8pool.tile([D, NP8, 2, NOUT], f8, name='w8t') if NP8 else None

            def issue_unit_dmas(u):
                if u == 8:
                    dmae.dma_start(bias[:], bias_ext[:])
                if NP8 and u == 16:
                    dmae.dma_start(w8t[:], w8_ext[:])
                if NP8 and u == max(NBF - 10, 2):
                    rt = r8pool.tile([D, 2 * r8_half * BC], bf, name="r8", tag="r8")
                    dmae.dma_start(rt[:], rots_ext[:, NRBF:NRBF + 2 * r8_half, :])
                    r8_tiles[0] = rt
                if NP8 and u == max(NBF - 5, 3):
                    n2 = 2 * (NP8 - r8_half)
                    rt = r8pool.tile([D, n2 * BC], bf, name="r8b", tag="r8b")
                    dmae.dma_start(
                        rt[:], rots_ext[:, NRBF + 2 * r8_half:NRBF + 2 * r8_half + n2, :])
                    r8_tiles[1] = rt
                if u < NBF:
                    g = wg_of_chunk[u]
                    if g not in wg_tiles and u == wg_starts[g]:
                        sz = wg_sizes[g]
                        wg = wpool.tile([D, sz * NOUT], bf, name="wg", tag="wg")
                        dmae.dma_start(wg[:], wp_ext[:, u:u + sz, :])
                        wg_tiles[g] = wg
                    rc = u - 2
                    if rc >= 0:
                        rg = rg_of_rc[rc]
                        if rg not in rg_tiles and rc == rg_starts[rg]:
                            sz = rg_sizes[rg]
                            rt = rpool.tile([D, sz * BC], bf, name="rg", tag="rg")
                            dmae.dma_start(rt[:], rots_ext[:, rc:rc + sz, :])
                            rg_tiles[rg] = rt

            def compute_product(u):
                if u == 0:
                    prod[0] = (xT, False)
                    return
                if u < NBF:
                    pt = pbf.tile([D, BC], bf, name="pt", tag="pt")
                    if u == 1:
                        nc.vector.tensor_mul(pt[:, 0:h], xT[:, 0:h], xT[:, 0:h])
                        nc.vector.tensor_mul(pt[:, h:BC], xT[:, h:BC], xT[:, h:BC])
                    else:
                        rc = u - 2
                        g = rg_of_rc[rc]
                        roff = rc - rg_starts[g]
                        rt = rg_tiles[g]
                        nc.vector.tensor_mul(
                            pt[:], xT[:], rt[:, roff * BC:(roff + 1) * BC])
                    prod[u] = (pt, False)
                else:
                    j = u - NBF
                    g = 0 if j < r8_half else 1
                    joff = j - g * r8_half
                    rt = r8_tiles[g]
                    pt = pf8.tile([D, 2, BC], f8, name="p8", tag="p8")
                    for k in range(2):
                        rcol = (2 * joff + k) * BC
                        nc.vector.tensor_mul(
                            pt[:, k, :], xT[:], rt[:, rcol:rcol + BC])
                    prod[u] = (pt, True)

            def release_unit(u):
                # last consumer done: allow pool rings to recycle
                prod.pop(u, None)
                if u < NBF:
                    g = wg_of_chunk[u]
                    if u == wg_starts[g] + wg_sizes[g] - 1:
                        wg_tiles.pop(g, None)

            def emit_matmul(u, i):
                b, n = i // NN, i % NN
                if u < NBF:
                    g = wg_of_chunk[u]
                    wg = wg_tiles[g]
                    woff = (u - wg_starts[g]) * NOUT
                    mv, _ = prod[u]
                    nc.tensor.matmul(
                        ps[i][:],
                        wg[:, woff + n * 128:woff + (n + 1) * 128],
                        mv[:, b * 512:(b + 1) * 512],
                        start=(u == 0),
                        stop=(u == NU - 1),
                    )
                else:
                    j = u - NBF
                    pt, _ = prod[u]
                    nc.tensor.matmul(
                        ps[i][:],
                        w8t[:, j, :, n * 128:(n + 1) * 128],
                        pt[:, :, b * 512:(b + 1) * 512],
                        start=False,
                        stop=(u == NU - 1),
                        perf_mode=mybir.MatmulPerfMode.DoubleRow,
                    )

            def drain_bank(i):
                b, n = i // NN, i % NN
                ot = opool.tile([D, 512], bf, name=f"ot_{i}", tag=f"ot_{i}")
                if i % 2 == 0:
                    nc.scalar.activation(
                        ot[:], ps[i][:],
                        mybir.ActivationFunctionType.Identity,
                        bias=bias[:, n:n + 1],
                    )
                else:
                    nc.vector.tensor_scalar_add(ot[:], ps[i][:], bias[:, n:n + 1])
                nc.sync.dma_start(
                    out_ext[n * 128:(n + 1) * 128, b * 512:(b + 1) * 512],
                    ot[:],
                )

            # wavefront: bank i processes unit u at step s = u + i
            for s in range(NU + NBANK - 1):
                if s < NU:
                    issue_unit_dmas(s)
                    compute_product(s)
                for i in range(NBANK):
                    u = s - i
                    if 0 <= u < NU:
                        emit_matmul(u, i)
                u_done = s - (NBANK - 1)
                if u_done >= 0:
                    release_unit(u_done)
                if s >= NU - 1:
                    drain_bank(s - (NU - 1))

    nc.compile()
    _nc_cache = nc
    return nc


def _prep_inputs(x, weights, bias):
    import ml_dtypes
    bf = np.dtype(ml_dtypes.bfloat16)
    f8 = np.dtype(ml_dtypes.float8_e4m3)

    x = np.asarray(x, dtype=np.float32)
    weights = np.asarray(weights, dtype=np.float32)
    bias = np.asarray(bias, dtype=np.float32)

    idx = _pair_index_map()
    wcols = weights.T  # [8384, 512]
    wp = np.zeros((NCHUNK, D, NOUT), dtype=np.float32)
    valid = idx >= 0
    wp[valid] = wcols[idx[valid]]
    wp_bf = np.ascontiguousarray(wp[:NBF].transpose(1, 0, 2)).astype(bf)
    if NP8:
        w8 = wp[NBF:].reshape(NP8, 2, D, NOUT).transpose(2, 0, 1, 3)
        w8 = np.ascontiguousarray(w8).astype(f8)

    biasp = np.ascontiguousarray(bias.reshape(NN, 128).T)  # [128, NN] f32

    in_maps = []
    for k in range(NCORES):
        xs = np.ascontiguousarray(x[k * BC:(k + 1) * BC].T).astype(bf)  # [128, BC]
        rots = np.stack([np.roll(xs, -d, axis=0) for d in range(1, NROT + 1)])
        rots = np.ascontiguousarray(rots.transpose(1, 0, 2))  # [D, NROT, BC]
        m = {"xT": xs, "rots": rots, "wp": wp_bf, "biasp": biasp}
        if NP8:
            m["w8"] = w8
        in_maps.append(m)
    return in_maps


def kernel(x, weights, bias):
    _ensure_axon_hooks_stub()
    from concourse.bass_utils import run_bass_kernel_spmd

    nc = _build_nc()
    in_maps = _prep_inputs(x, weights, bias)
    res = run_bass_kernel_spmd(nc, in_maps, core_ids=list(range(NCORES)))
    outT = np.concatenate(
        [np.asarray(res.results[k]["out"], dtype=np.float32) for k in range(NCORES)],
        axis=1,
    )
    out = np.ascontiguousarray(outT.T, dtype=np.float32)  # [8192, 512]
    kernel.last_results = res
    return out
